# revision 19
# baseline (speedup 1.0000x reference)
"""A3TGCN2 Trainium2 kernel — 8-core node-partitioned Bass implementation.

Math (reference has H0 == 0 throughout, so R is irrelevant and timesteps
are independent):
    out[b] = sum_t p_t * (1 - Z_t) * Htld_t
    Z_t    = sigmoid((A @ x_bt) @ Wz' + bz'),  Htld_t = tanh((A @ x_bt) @ Wh' + bh')
    A      = sym-normalized adjacency with self loops.
Since A @ (x @ W) == (A @ x) @ W, we aggregate the raw [N, B*F*T=96]-wide
features once (sparse scatter-add), then run tiny dense matmuls per batch.

Device layout per core (nodes partitioned, 6250/core):
  - gather X rows per edge via gpsimd.dma_gather (bf16 [N,128] tables,
    split into two <32768-row halves for int16 indices)
  - msg = norm_e * X[row_e] (DVE, norm precomputed on host incl. both dinvs)
  - Y^T psum [96,128] += msg^T-matmul with one-hot selection rhs (TensorE)
  - dense: [25,128]x[25,384] matmuls (24 feats + ones row for bias) -> psum
  - ACT sigmoid/tanh (Wz negated so sigmoid yields 1-Z), DVE combine with
    softmax(attention) weights, DMA out.
"""

import sys

sys.path.insert(0, "/opt/trn_rl_repo")

import numpy as np
import ml_dtypes

import concourse.bacc as bacc
import concourse.bass as bass
import concourse.mybir as mybir
import concourse.tile as tile
from concourse.bass_utils import run_bass_kernel_spmd
from concourse.library_config import mlp

BF16 = mybir.dt.bfloat16
FP32 = mybir.dt.float32
I16 = mybir.dt.int16

B, N, F, T, O = 4, 50000, 2, 12, 32
E = 800000
NC = 8
NPC = N // NC            # 6250 nodes per core
G = (NPC + 127) // 128   # 49 dst groups per core
NPC_PAD = G * 128        # 6272
HALF = 25000             # int16 index limit split
SG_GROUPS = 4            # dst groups per gather super-group
FEAT = B * F * T         # 96

_CACHE = {}


def _pack_idx(idx, total):
    """Wrap int16 indices: idx j -> [j%16, j//16]; returns [128, total//16]."""
    out = np.zeros((16, total // 16), dtype=np.int16)
    j = np.arange(len(idx))
    out[j % 16, j // 16] = idx
    return np.tile(out, (8, 1))  # replicate across the 8 Q7 cores' blocks


def _build_host(X, edge_index, edge_weight, W_z, b_z, W_h, b_h,
                lin_z_w, lin_z_b, lin_h_w, lin_h_b, attention):
    """All host-side prep: norm, partitioning, padding, weight folding."""
    row = np.asarray(edge_index[0], dtype=np.int64)
    col = np.asarray(edge_index[1], dtype=np.int64)
    w = np.asarray(edge_weight, dtype=np.float64)

    deg = np.bincount(col, weights=w, minlength=N) + 1.0
    dinv = 1.0 / np.sqrt(deg)
    norm = (dinv[row] * w * dinv[col]).astype(np.float32)
    selfw = (dinv * dinv).astype(np.float32)

    # X tables [N, 128] bf16 (row = node, cols 0:96 = (b,f,t), rest 0)
    Xr = np.moveaxis(np.asarray(X, np.float32), 1, 0).reshape(N, FEAT)
    Xtab = np.zeros((N, 128), dtype=ml_dtypes.bfloat16)
    Xtab.reshape(N, B, 32)[:, :, :F * T] = Xr.reshape(N, B, F * T).astype(
        ml_dtypes.bfloat16)
    X_lo, X_hi = Xtab[:HALF], Xtab[HALF:]

    # per-core edge grouping
    per_core = []
    for c in range(NC):
        base = c * NPC
        m = (col >= base) & (col < base + NPC)
        er, ec, en = row[m], col[m] - base, norm[m]
        g = ec // 128
        half = (er >= HALF).astype(np.int64)
        order = np.lexsort((er, half, g))
        er, ec, en, g, half = er[order], ec[order], en[order], g[order], half[order]
        per_core.append((er, ec, en, g, half))

    # tile counts per (group, half): max over cores, >=1
    LT = np.ones(G, dtype=np.int64)
    HT = np.ones(G, dtype=np.int64)
    for er, ec, en, g, half in per_core:
        for gg in range(G):
            sel = g == gg
            nlo = int(((half == 0) & sel).sum())
            nhi = int(((half == 1) & sel).sum())
            LT[gg] = max(LT[gg], (nlo + 127) // 128)
            HT[gg] = max(HT[gg], (nhi + 127) // 128)

    # super-groups
    sg_bounds = [(s, min(s + SG_GROUPS, G)) for s in range(0, G, SG_GROUPS)]

    # slot map: per SG -> [lo tiles g0..gk | hi tiles g0..gk | self g0..gk]
    slot_lo, slot_hi, slot_self, sg_base, sg_tiles = {}, {}, {}, [], []
    tot_tiles = 0
    for (s, e) in sg_bounds:
        sg_base.append(tot_tiles)
        off = tot_tiles
        for gg in range(s, e):
            slot_lo[gg] = off
            off += LT[gg]
        for gg in range(s, e):
            slot_hi[gg] = off
            off += HT[gg]
        for gg in range(s, e):
            slot_self[gg] = off
            off += 1
        sg_tiles.append(off - tot_tiles)
        tot_tiles = off
    n_lo = {gg: int(LT[gg]) for gg in range(G)}
    n_hi = {gg: int(HT[gg]) for gg in range(G)}

    # per-SG gather index segment sizes (same all cores)
    sg_nlo = [int(LT[s:e].sum()) * 128 for (s, e) in sg_bounds]
    sg_nhi = [int(HT[s:e].sum()) * 128 for (s, e) in sg_bounds]
    idxlo_tot, idxhi_tot = sum(sg_nlo), sum(sg_nhi)

    # per-core packed arrays
    colrel_all = np.zeros((NC, 128, tot_tiles), dtype=ml_dtypes.bfloat16)
    w_all = np.zeros((NC, 128, tot_tiles), dtype=ml_dtypes.bfloat16)
    idxlo_all = np.zeros((NC, 128, idxlo_tot // 16), dtype=np.int16)
    idxhi_all = np.zeros((NC, 128, idxhi_tot // 16), dtype=np.int16)
    xself_all = np.zeros((NC, NPC_PAD, 128), dtype=ml_dtypes.bfloat16)

    for c in range(NC):
        er, ec, en, g, half = per_core[c]
        base = c * NPC
        xself_all[c, :NPC] = Xtab[base:base + NPC]
        lo_seg_off, hi_seg_off = 0, 0
        lo_parts, hi_parts = [], []
        for sgi, (s, e) in enumerate(sg_bounds):
            lo_idx_seg, hi_idx_seg = [], []
            for gg in range(s, e):
                for hv, slot0, ntile, seg in (
                    (0, slot_lo[gg], n_lo[gg], lo_idx_seg),
                    (1, slot_hi[gg], n_hi[gg], hi_idx_seg),
                ):
                    sel = (g == gg) & (half == hv)
                    r, cc, nn = er[sel], ec[sel] % 128, en[sel]
                    npad = ntile * 128 - len(r)
                    r = np.concatenate([r - hv * HALF, np.zeros(npad, np.int64)])
                    cc = np.concatenate([cc, np.zeros(npad, np.int64)])
                    nn = np.concatenate([nn, np.zeros(npad, np.float32)])
                    seg.append(r)
                    for tt in range(ntile):
                        sl = slice(tt * 128, (tt + 1) * 128)
                        colrel_all[c, :, slot0 + tt] = cc[sl].astype(
                            ml_dtypes.bfloat16)
                        w_all[c, :, slot0 + tt] = nn[sl].astype(ml_dtypes.bfloat16)
                # self tile
                ss = slot_self[gg]
                nrows = min(128, NPC - gg * 128)
                cc = np.arange(128) % 128
                ww = np.zeros(128, np.float32)
                ww[:nrows] = selfw[base + gg * 128: base + gg * 128 + nrows]
                colrel_all[c, :, ss] = cc.astype(ml_dtypes.bfloat16)
                w_all[c, :, ss] = ww.astype(ml_dtypes.bfloat16)
            lo_parts.append(_pack_idx(np.concatenate(lo_idx_seg), sg_nlo[sgi]))
            hi_parts.append(_pack_idx(np.concatenate(hi_idx_seg), sg_nhi[sgi]))
        idxlo_all[c] = np.concatenate(lo_parts, axis=1)
        idxhi_all[c] = np.concatenate(hi_parts, axis=1)

    # folded weights (Wz negated -> sigmoid gives 1-Z directly)
    Wz = -(np.asarray(W_z, np.float64) @ np.asarray(lin_z_w, np.float64)[:O])
    bz = -(np.asarray(b_z, np.float64) @ np.asarray(lin_z_w, np.float64)[:O]
           + np.asarray(lin_z_b, np.float64))
    Wh = np.asarray(W_h, np.float64) @ np.asarray(lin_h_w, np.float64)[:O]
    bh = (np.asarray(b_h, np.float64) @ np.asarray(lin_h_w, np.float64)[:O]
          + np.asarray(lin_h_b, np.float64))
    a = np.asarray(attention, np.float64)
    p = np.exp(a - a.max())
    p /= p.sum()

    # rhs [25, 384]: rows k=f*12+t -> block-diag W[f,:] at cols t*32..; row 24 = bias
    WZr = np.zeros((25, T * O), dtype=ml_dtypes.bfloat16)
    WHr = np.zeros((25, T * O), dtype=ml_dtypes.bfloat16)
    for f in range(F):
        for t in range(T):
            WZr[f * T + t, t * O:(t + 1) * O] = Wz[f].astype(ml_dtypes.bfloat16)
            WHr[f * T + t, t * O:(t + 1) * O] = Wh[f].astype(ml_dtypes.bfloat16)
    WZr[24] = np.tile(bz, T).astype(ml_dtypes.bfloat16)
    WHr[24] = np.tile(bh, T).astype(ml_dtypes.bfloat16)

    pvec = np.repeat(p, O).astype(ml_dtypes.bfloat16)       # [384]
    pv_tile = np.broadcast_to(pvec, (128, T * O)).copy()
    iota = np.broadcast_to(np.arange(128, dtype=np.float32),
                           (128, 128)).astype(ml_dtypes.bfloat16).copy()

    meta = dict(sg_bounds=sg_bounds, sg_base=sg_base, sg_tiles=sg_tiles,
                sg_nlo=sg_nlo, sg_nhi=sg_nhi, slot_lo=slot_lo, slot_hi=slot_hi,
                slot_self=slot_self, n_lo=n_lo, n_hi=n_hi, tot_tiles=tot_tiles,
                idxlo_tot=idxlo_tot, idxhi_tot=idxhi_tot)
    consts = dict(X_lo=np.ascontiguousarray(X_lo), X_hi=np.ascontiguousarray(X_hi),
                  WZ=WZr, WH=WHr, PV=pv_tile, IOTA=iota,
                  ONES=np.ones((1, 128), dtype=ml_dtypes.bfloat16))
    per_core_in = [dict(XSELF=xself_all[c], COLREL=colrel_all[c], WSC=w_all[c],
                        IDXLO=idxlo_all[c], IDXHI=idxhi_all[c]) for c in range(NC)]
    return meta, consts, per_core_in


def _build_graph(meta):
    nc = bacc.Bacc("TRN2", target_bir_lowering=False)
    tt = meta["tot_tiles"]

    x_lo = nc.dram_tensor("X_lo", [HALF, 128], BF16, kind="ExternalInput")
    x_hi = nc.dram_tensor("X_hi", [HALF, 128], BF16, kind="ExternalInput")
    x_self = nc.dram_tensor("XSELF", [NPC_PAD, 128], BF16, kind="ExternalInput")
    colrel = nc.dram_tensor("COLREL", [128, tt], BF16, kind="ExternalInput")
    wsc = nc.dram_tensor("WSC", [128, tt], BF16, kind="ExternalInput")
    idxlo = nc.dram_tensor("IDXLO", [128, meta["idxlo_tot"] // 16], I16,
                           kind="ExternalInput")
    idxhi = nc.dram_tensor("IDXHI", [128, meta["idxhi_tot"] // 16], I16,
                           kind="ExternalInput")
    wz = nc.dram_tensor("WZ", [25, T * O], BF16, kind="ExternalInput")
    wh = nc.dram_tensor("WH", [25, T * O], BF16, kind="ExternalInput")
    pv = nc.dram_tensor("PV", [128, T * O], BF16, kind="ExternalInput")
    iota = nc.dram_tensor("IOTA", [128, 128], BF16, kind="ExternalInput")
    ones = nc.dram_tensor("ONES", [1, 128], BF16, kind="ExternalInput")
    out = nc.dram_tensor("out", [B, NPC_PAD, O], FP32, kind="ExternalOutput")

    sgb = meta["sg_bounds"]
    n_sg = len(sgb)
    max_sg_tiles = max(meta["sg_tiles"])

    from contextlib import ExitStack

    with tile.TileContext(nc) as tc, ExitStack() as ctx:
        nc.gpsimd.load_library(mlp)
        ec = ctx.enter_context
        static_tp = ec(tc.tile_pool(name="static", bufs=1))
        sb_colrel = static_tp.tile([128, tt], BF16)
        sb_w = static_tp.tile([128, tt], BF16)
        sb_idxlo = static_tp.tile([128, meta["idxlo_tot"] // 16], I16)
        sb_idxhi = static_tp.tile([128, meta["idxhi_tot"] // 16], I16)
        sb_wz = static_tp.tile([25, T * O], BF16)
        sb_wh = static_tp.tile([25, T * O], BF16)
        sb_pv = static_tp.tile([128, T * O], BF16)
        sb_iota = static_tp.tile([128, 128], BF16)
        ytn = [[static_tp.tile([25, 128], BF16, name=f"yt{p}{b}")
                for b in range(B)] for p in range(2)]
        gath_tp = ec(tc.tile_pool(name="gath", bufs=2))
        sel_tp = ec(tc.tile_pool(name="sel", bufs=2))
        zh_tp = ec(tc.tile_pool(name="zh", bufs=2))
        osb_tp = ec(tc.tile_pool(name="osb", bufs=2))
        pyt_tp = ec(tc.tile_pool(name="pyt", bufs=2, space="PSUM"))
        pzh_tp = ec(tc.tile_pool(name="pzh", bufs=2, space="PSUM"))
        if True:
            nc.sync.dma_start(sb_colrel[:], colrel[:])
            nc.sync.dma_start(sb_w[:], wsc[:])
            nc.sync.dma_start(sb_idxlo[:], idxlo[:])
            nc.sync.dma_start(sb_idxhi[:], idxhi[:])
            nc.sync.dma_start(sb_wz[:], wz[:])
            nc.sync.dma_start(sb_wh[:], wh[:])
            nc.sync.dma_start(sb_pv[:], pv[:])
            nc.sync.dma_start(sb_iota[:], iota[:])
            yts = ytn
            for par in yts:
                for yt in par:
                    nc.sync.dma_start(yt[24:25, :], ones[:])

            lo_off = hi_off = 0
            for sgi, (s, e) in enumerate(sgb):
                base = meta["sg_base"][sgi]
                ntile = meta["sg_tiles"][sgi]
                nlo, nhi = meta["sg_nlo"][sgi], meta["sg_nhi"][sgi]
                gbuf = gath_tp.tile([128, max_sg_tiles, 128], BF16)

                nc.gpsimd.dma_gather(
                    gbuf[:, : nlo // 128, :], x_lo[:],
                    sb_idxlo[:, lo_off: lo_off + nlo // 16], nlo, nlo, 128,
                    single_packet=False)
                nc.gpsimd.dma_gather(
                    gbuf[:, nlo // 128: (nlo + nhi) // 128, :], x_hi[:],
                    sb_idxhi[:, hi_off: hi_off + nhi // 16], nhi, nhi, 128,
                    single_packet=False)
                lo_off += nlo // 16
                hi_off += nhi // 16
                for gg in range(s, e):
                    sslot = meta["slot_self"][gg] - base
                    nc.sync.dma_start(
                        gbuf[:, sslot, :],
                        x_self[gg * 128:(gg + 1) * 128, :])

                # batched S build + w-scale for whole SG
                smat = sel_tp.tile([128, max_sg_tiles, 128], BF16)
                nc.vector.tensor_tensor(
                    out=smat[:, :ntile, :],
                    in0=sb_colrel[:, base:base + ntile, None].to_broadcast(
                        [128, ntile, 128]),
                    in1=sb_iota[:, None, :].to_broadcast([128, ntile, 128]),
                    op=mybir.AluOpType.is_equal)
                nc.vector.tensor_tensor(
                    out=gbuf[:, :ntile, :],
                    in0=gbuf[:, :ntile, :],
                    in1=sb_w[:, base:base + ntile, None].to_broadcast(
                        [128, ntile, 128]),
                    op=mybir.AluOpType.mult)

                for gg in range(s, e):
                    ytb = yts[gg % 2]
                    pyt = pyt_tp.tile([128, 128], FP32, space="PSUM")
                    slots = (
                        list(range(meta["slot_lo"][gg] - base,
                                   meta["slot_lo"][gg] - base + meta["n_lo"][gg]))
                        + list(range(meta["slot_hi"][gg] - base,
                                     meta["slot_hi"][gg] - base + meta["n_hi"][gg]))
                        + [meta["slot_self"][gg] - base])
                    for i, sl in enumerate(slots):
                        nc.tensor.matmul(
                            out=pyt[:], lhsT=gbuf[:, sl, :],
                            rhs=smat[:, sl, :],
                            start=(i == 0), stop=(i == len(slots) - 1))
                    for b in range(B):
                        nc.scalar.activation(
                            out=ytb[b][0:24, :],
                            in_=pyt[b * 32: b * 32 + 24, :],
                            func=mybir.ActivationFunctionType.Copy)

                    zc = zh_tp.tile([128, B, T * O], BF16)
                    ht = zh_tp.tile([128, B, T * O], BF16)
                    for b in range(B):
                        pz = pzh_tp.tile([128, T * O], FP32, space="PSUM")
                        ph = pzh_tp.tile([128, T * O], FP32, space="PSUM")
                        nc.tensor.matmul(out=pz[:], lhsT=ytb[b][:, :],
                                         rhs=sb_wz[:], start=True, stop=True)
                        nc.tensor.matmul(out=ph[:], lhsT=ytb[b][:, :],
                                         rhs=sb_wh[:], start=True, stop=True)
                        nc.scalar.activation(
                            out=zc[:, b, :], in_=pz[:],
                            func=mybir.ActivationFunctionType.Sigmoid)
                        nc.scalar.activation(
                            out=ht[:, b, :], in_=ph[:],
                            func=mybir.ActivationFunctionType.Tanh)
                    # comb = zc*pv -> *ht -> tree-reduce over t
                    nc.vector.tensor_tensor(
                        out=zc[:], in0=zc[:],
                        in1=sb_pv[:, None, :].to_broadcast([128, B, T * O]),
                        op=mybir.AluOpType.mult)
                    nc.vector.tensor_tensor(out=zc[:], in0=zc[:], in1=ht[:],
                                            op=mybir.AluOpType.mult)
                    nc.vector.tensor_tensor(
                        out=zc[:, :, 0:192], in0=zc[:, :, 0:192],
                        in1=zc[:, :, 192:384], op=mybir.AluOpType.add)
                    nc.vector.tensor_tensor(
                        out=zc[:, :, 0:96], in0=zc[:, :, 0:96],
                        in1=zc[:, :, 96:192], op=mybir.AluOpType.add)
                    nc.vector.tensor_tensor(
                        out=zc[:, :, 0:32], in0=zc[:, :, 0:32],
                        in1=zc[:, :, 32:64], op=mybir.AluOpType.add)
                    osb = osb_tp.tile([128, B, O], FP32)
                    nc.vector.tensor_tensor(
                        out=osb[:], in0=zc[:, :, 0:32], in1=zc[:, :, 64:96],
                        op=mybir.AluOpType.add)
                    for b in range(B):
                        nc.sync.dma_start(
                            out[b, gg * 128:(gg + 1) * 128, :], osb[:, b, :])
    nc.compile()
    return nc


def kernel(**inputs):
    X = np.asarray(inputs["X"])
    key = "k"
    meta, consts, per_core_in = _build_host(
        X, inputs["edge_index"], inputs["edge_weight"],
        inputs["W_z"], inputs["b_z"], inputs["W_h"], inputs["b_h"],
        inputs["lin_z_w"], inputs["lin_z_b"], inputs["lin_h_w"],
        inputs["lin_h_b"], inputs["attention"])

    if key not in _CACHE:
        _CACHE[key] = _build_graph(meta)
    nc = _CACHE[key]

    in_maps = []
    for c in range(NC):
        m = dict(consts)
        m.update(per_core_in[c])
        in_maps.append({k: np.ascontiguousarray(v) for k, v in m.items()})
    res = run_bass_kernel_spmd(nc, in_maps, core_ids=list(range(NC)))
    _CACHE["last_results"] = res
    full = np.empty((B, N, O), dtype=np.float32)
    for c in range(NC):
        full[:, c * NPC:(c + 1) * NPC, :] = res.results[c]["out"][:, :NPC, :]
    return full


# revision 23
# speedup vs baseline: 1.7304x; 1.7304x over previous
"""A3TGCN2 Trainium2 kernel — 8-core node-partitioned Bass implementation.

Math (reference has H0 == 0 throughout, so R is irrelevant and timesteps
are independent):
    out[b] = sum_t p_t * (1 - Z_t) * Htld_t
    Z_t    = sigmoid((A @ x_bt) @ Wz' + bz'),  Htld_t = tanh((A @ x_bt) @ Wh' + bh')
    A      = sym-normalized adjacency with self loops.
Since A @ (x @ W) == (A @ x) @ W, we aggregate the raw [N, B*F*T=96]-wide
features once (sparse scatter-add), then run tiny dense matmuls per batch.

Device layout per core (nodes partitioned, 6250/core):
  - gather X rows per edge via gpsimd.dma_gather (bf16 [N,128] tables,
    split into two <32768-row halves for int16 indices)
  - msg = norm_e * X[row_e] (DVE, norm precomputed on host incl. both dinvs)
  - Y^T psum [96,128] += msg^T-matmul with one-hot selection rhs (TensorE)
  - dense: [25,128]x[25,384] matmuls (24 feats + ones row for bias) -> psum
  - ACT sigmoid/tanh (Wz negated so sigmoid yields 1-Z), DVE combine with
    softmax(attention) weights, DMA out.
"""

import sys

sys.path.insert(0, "/opt/trn_rl_repo")

import numpy as np
import ml_dtypes

import concourse.bacc as bacc
import concourse.bass as bass
import concourse.mybir as mybir
import concourse.tile as tile
from concourse.bass_utils import run_bass_kernel_spmd
from concourse.library_config import mlp

BF16 = mybir.dt.bfloat16
FP32 = mybir.dt.float32
I16 = mybir.dt.int16

B, N, F, T, O = 4, 50000, 2, 12, 32
E = 800000
NC = 8
NPC = N // NC            # 6250 nodes per core
G = (NPC + 127) // 128   # 49 dst groups per core
NPC_PAD = G * 128        # 6272
HALF = 25000             # int16 index limit split
SG_GROUPS = 4            # dst groups per gather super-group
FEAT = B * F * T         # 96

_CACHE = {}


def _pack_idx(idx, total):
    """Wrap int16 indices: idx j -> [j%16, j//16]; returns [128, total//16]."""
    out = np.zeros((16, total // 16), dtype=np.int16)
    j = np.arange(len(idx))
    out[j % 16, j // 16] = idx
    return np.tile(out, (8, 1))  # replicate across the 8 Q7 cores' blocks


def _build_host(X, edge_index, edge_weight, W_z, b_z, W_h, b_h,
                lin_z_w, lin_z_b, lin_h_w, lin_h_b, attention):
    """All host-side prep: norm, partitioning, padding, weight folding."""
    row = np.asarray(edge_index[0], dtype=np.int64)
    col = np.asarray(edge_index[1], dtype=np.int64)
    w = np.asarray(edge_weight, dtype=np.float64)

    deg = np.bincount(col, weights=w, minlength=N) + 1.0
    dinv = 1.0 / np.sqrt(deg)
    norm = (dinv[row] * w * dinv[col]).astype(np.float32)
    selfw = (dinv * dinv).astype(np.float32)

    # X tables [N, 128] bf16 (row = node, cols 0:96 = (b,f,t), rest 0)
    Xr = np.moveaxis(np.asarray(X, np.float32), 1, 0).reshape(N, FEAT)
    Xtab = np.zeros((N, 128), dtype=ml_dtypes.bfloat16)
    Xtab.reshape(N, B, 32)[:, :, :F * T] = Xr.reshape(N, B, F * T).astype(
        ml_dtypes.bfloat16)
    X_lo, X_hi = Xtab[:HALF], Xtab[HALF:]

    # per-core edge grouping
    per_core = []
    for c in range(NC):
        base = c * NPC
        m = (col >= base) & (col < base + NPC)
        er, ec, en = row[m], col[m] - base, norm[m]
        g = ec // 128
        half = (er >= HALF).astype(np.int64)
        order = np.lexsort((er, half, g))
        er, ec, en, g, half = er[order], ec[order], en[order], g[order], half[order]
        per_core.append((er, ec, en, g, half))

    # tile counts per (group, half): max over cores, >=1
    LT = np.ones(G, dtype=np.int64)
    HT = np.ones(G, dtype=np.int64)
    for er, ec, en, g, half in per_core:
        for gg in range(G):
            sel = g == gg
            nlo = int(((half == 0) & sel).sum())
            nhi = int(((half == 1) & sel).sum())
            LT[gg] = max(LT[gg], (nlo + 127) // 128)
            HT[gg] = max(HT[gg], (nhi + 127) // 128)

    # super-groups
    sg_bounds = [(s, min(s + SG_GROUPS, G)) for s in range(0, G, SG_GROUPS)]

    # slot map: per SG -> [lo tiles g0..gk | hi tiles g0..gk | self g0..gk]
    slot_lo, slot_hi, slot_self, sg_base, sg_tiles = {}, {}, {}, [], []
    tot_tiles = 0
    for (s, e) in sg_bounds:
        sg_base.append(tot_tiles)
        off = tot_tiles
        for gg in range(s, e):
            slot_lo[gg] = off
            off += LT[gg]
        for gg in range(s, e):
            slot_hi[gg] = off
            off += HT[gg]
        for gg in range(s, e):
            slot_self[gg] = off
            off += 1
        sg_tiles.append(off - tot_tiles)
        tot_tiles = off
    n_lo = {gg: int(LT[gg]) for gg in range(G)}
    n_hi = {gg: int(HT[gg]) for gg in range(G)}

    # per-SG gather index segment sizes (same all cores)
    sg_nlo = [int(LT[s:e].sum()) * 128 for (s, e) in sg_bounds]
    sg_nhi = [int(HT[s:e].sum()) * 128 for (s, e) in sg_bounds]
    idxlo_tot, idxhi_tot = sum(sg_nlo), sum(sg_nhi)

    # per-core packed arrays
    colrel_all = np.zeros((NC, 128, tot_tiles), dtype=ml_dtypes.bfloat16)
    w_all = np.zeros((NC, 128, tot_tiles), dtype=ml_dtypes.bfloat16)
    idxlo_all = np.zeros((NC, 128, idxlo_tot // 16), dtype=np.int16)
    idxhi_all = np.zeros((NC, 128, idxhi_tot // 16), dtype=np.int16)
    xself_all = np.zeros((NC, NPC_PAD, 128), dtype=ml_dtypes.bfloat16)

    for c in range(NC):
        er, ec, en, g, half = per_core[c]
        base = c * NPC
        xself_all[c, :NPC] = Xtab[base:base + NPC]
        lo_seg_off, hi_seg_off = 0, 0
        lo_parts, hi_parts = [], []
        for sgi, (s, e) in enumerate(sg_bounds):
            lo_idx_seg, hi_idx_seg = [], []
            for gg in range(s, e):
                for hv, slot0, ntile, seg in (
                    (0, slot_lo[gg], n_lo[gg], lo_idx_seg),
                    (1, slot_hi[gg], n_hi[gg], hi_idx_seg),
                ):
                    sel = (g == gg) & (half == hv)
                    r, cc, nn = er[sel], ec[sel] % 128, en[sel]
                    npad = ntile * 128 - len(r)
                    r = np.concatenate([r - hv * HALF, np.zeros(npad, np.int64)])
                    cc = np.concatenate([cc, np.zeros(npad, np.int64)])
                    nn = np.concatenate([nn, np.zeros(npad, np.float32)])
                    seg.append(r)
                    for tt in range(ntile):
                        sl = slice(tt * 128, (tt + 1) * 128)
                        colrel_all[c, :, slot0 + tt] = cc[sl].astype(
                            ml_dtypes.bfloat16)
                        w_all[c, :, slot0 + tt] = nn[sl].astype(ml_dtypes.bfloat16)
                # self tile
                ss = slot_self[gg]
                nrows = min(128, NPC - gg * 128)
                cc = np.arange(128) % 128
                ww = np.zeros(128, np.float32)
                ww[:nrows] = selfw[base + gg * 128: base + gg * 128 + nrows]
                colrel_all[c, :, ss] = cc.astype(ml_dtypes.bfloat16)
                w_all[c, :, ss] = ww.astype(ml_dtypes.bfloat16)
            lo_parts.append(_pack_idx(np.concatenate(lo_idx_seg), sg_nlo[sgi]))
            hi_parts.append(_pack_idx(np.concatenate(hi_idx_seg), sg_nhi[sgi]))
        idxlo_all[c] = np.concatenate(lo_parts, axis=1)
        idxhi_all[c] = np.concatenate(hi_parts, axis=1)

    # folded weights (Wz negated -> sigmoid gives 1-Z directly)
    Wz = -(np.asarray(W_z, np.float64) @ np.asarray(lin_z_w, np.float64)[:O])
    bz = -(np.asarray(b_z, np.float64) @ np.asarray(lin_z_w, np.float64)[:O]
           + np.asarray(lin_z_b, np.float64))
    Wh = np.asarray(W_h, np.float64) @ np.asarray(lin_h_w, np.float64)[:O]
    bh = (np.asarray(b_h, np.float64) @ np.asarray(lin_h_w, np.float64)[:O]
          + np.asarray(lin_h_b, np.float64))
    a = np.asarray(attention, np.float64)
    p = np.exp(a - a.max())
    p /= p.sum()

    # rhs [25, 384]: rows k=f*12+t -> block-diag W[f,:] at cols t*32..; row 24 = bias
    WZr = np.zeros((25, T * O), dtype=ml_dtypes.bfloat16)
    WHr = np.zeros((25, T * O), dtype=ml_dtypes.bfloat16)
    for f in range(F):
        for t in range(T):
            WZr[f * T + t, t * O:(t + 1) * O] = Wz[f].astype(ml_dtypes.bfloat16)
            WHr[f * T + t, t * O:(t + 1) * O] = Wh[f].astype(ml_dtypes.bfloat16)
    WZr[24] = np.tile(bz, T).astype(ml_dtypes.bfloat16)
    WHr[24] = np.tile(bh, T).astype(ml_dtypes.bfloat16)

    pvec = np.repeat(p, O).astype(ml_dtypes.bfloat16)       # [384]
    pv_tile = np.broadcast_to(pvec, (128, T * O)).copy()
    iota = np.broadcast_to(np.arange(128, dtype=np.float32),
                           (128, 128)).astype(ml_dtypes.bfloat16).copy()

    meta = dict(sg_bounds=sg_bounds, sg_base=sg_base, sg_tiles=sg_tiles,
                sg_nlo=sg_nlo, sg_nhi=sg_nhi, slot_lo=slot_lo, slot_hi=slot_hi,
                slot_self=slot_self, n_lo=n_lo, n_hi=n_hi, tot_tiles=tot_tiles,
                idxlo_tot=idxlo_tot, idxhi_tot=idxhi_tot)
    consts = dict(X_lo=np.ascontiguousarray(X_lo), X_hi=np.ascontiguousarray(X_hi),
                  WZ=WZr, WH=WHr, PV=pv_tile, IOTA=iota,
                  ONES=np.ones((1, 128), dtype=ml_dtypes.bfloat16))
    per_core_in = [dict(XSELF=xself_all[c], COLREL=colrel_all[c], WSC=w_all[c],
                        IDXLO=idxlo_all[c], IDXHI=idxhi_all[c]) for c in range(NC)]
    return meta, consts, per_core_in


def _build_graph(meta):
    nc = bacc.Bacc("TRN2", target_bir_lowering=False, num_swdge_queues=4)
    tt = meta["tot_tiles"]

    x_lo = nc.dram_tensor("X_lo", [HALF, 128], BF16, kind="ExternalInput")
    x_hi = nc.dram_tensor("X_hi", [HALF, 128], BF16, kind="ExternalInput")
    x_self = nc.dram_tensor("XSELF", [NPC_PAD, 128], BF16, kind="ExternalInput")
    colrel = nc.dram_tensor("COLREL", [128, tt], BF16, kind="ExternalInput")
    wsc = nc.dram_tensor("WSC", [128, tt], BF16, kind="ExternalInput")
    idxlo = nc.dram_tensor("IDXLO", [128, meta["idxlo_tot"] // 16], I16,
                           kind="ExternalInput")
    idxhi = nc.dram_tensor("IDXHI", [128, meta["idxhi_tot"] // 16], I16,
                           kind="ExternalInput")
    wz = nc.dram_tensor("WZ", [25, T * O], BF16, kind="ExternalInput")
    wh = nc.dram_tensor("WH", [25, T * O], BF16, kind="ExternalInput")
    pv = nc.dram_tensor("PV", [128, T * O], BF16, kind="ExternalInput")
    iota = nc.dram_tensor("IOTA", [128, 128], BF16, kind="ExternalInput")
    ones = nc.dram_tensor("ONES", [1, 128], BF16, kind="ExternalInput")
    out = nc.dram_tensor("out", [B, NPC_PAD, O], FP32, kind="ExternalOutput")

    sgb = meta["sg_bounds"]
    n_sg = len(sgb)
    max_sg_tiles = max(meta["sg_tiles"])

    from contextlib import ExitStack

    with tile.TileContext(nc) as tc, ExitStack() as ctx:
        nc.gpsimd.load_library(mlp)
        ec = ctx.enter_context
        static_tp = ec(tc.tile_pool(name="static", bufs=1))
        sb_colrel = static_tp.tile([128, tt], BF16)
        sb_w = static_tp.tile([128, tt], BF16)
        sb_idxlo = static_tp.tile([128, meta["idxlo_tot"] // 16], I16)
        sb_idxhi = static_tp.tile([128, meta["idxhi_tot"] // 16], I16)
        sb_wz = static_tp.tile([25, T * O], BF16)
        sb_wh = static_tp.tile([25, T * O], BF16)
        sb_pv = static_tp.tile([128, T * O], BF16)
        sb_iota = static_tp.tile([128, 128], BF16)
        ytn = [[static_tp.tile([25, 128], BF16, name=f"yt{p}{b}")
                for b in range(B)] for p in range(2)]
        gath_tp = ec(tc.tile_pool(name="gath", bufs=2))
        sel_tp = ec(tc.tile_pool(name="sel", bufs=2))
        zh_tp = ec(tc.tile_pool(name="zh", bufs=2))
        osb_tp = ec(tc.tile_pool(name="osb", bufs=2))
        pyt_tp = ec(tc.tile_pool(name="pyt", bufs=2, space="PSUM"))
        pzh_tp = ec(tc.tile_pool(name="pzh", bufs=2, space="PSUM"))
        if True:
            nc.sync.dma_start(sb_colrel[:], colrel[:])
            nc.sync.dma_start(sb_w[:], wsc[:])
            nc.sync.dma_start(sb_idxlo[:], idxlo[:])
            nc.sync.dma_start(sb_idxhi[:], idxhi[:])
            nc.sync.dma_start(sb_wz[:], wz[:])
            nc.sync.dma_start(sb_wh[:], wh[:])
            nc.sync.dma_start(sb_pv[:], pv[:])
            nc.sync.dma_start(sb_iota[:], iota[:])
            yts = ytn
            for par in yts:
                for yt in par:
                    nc.sync.dma_start(yt[24:25, :], ones[:])

            lo_off = hi_off = 0
            for sgi, (s, e) in enumerate(sgb):
                base = meta["sg_base"][sgi]
                ntile = meta["sg_tiles"][sgi]
                nlo, nhi = meta["sg_nlo"][sgi], meta["sg_nhi"][sgi]
                gbuf = gath_tp.tile([128, max_sg_tiles, 128], BF16)

                nc.gpsimd.dma_gather(
                    gbuf[:, : nlo // 128, :], x_lo[:],
                    sb_idxlo[:, lo_off: lo_off + nlo // 16], nlo, nlo, 128,
                    single_packet=False, queue_num=(2 * sgi) % 4)
                nc.gpsimd.dma_gather(
                    gbuf[:, nlo // 128: (nlo + nhi) // 128, :], x_hi[:],
                    sb_idxhi[:, hi_off: hi_off + nhi // 16], nhi, nhi, 128,
                    single_packet=False, queue_num=(2 * sgi + 1) % 4)
                lo_off += nlo // 16
                hi_off += nhi // 16
                for gg in range(s, e):
                    sslot = meta["slot_self"][gg] - base
                    nc.sync.dma_start(
                        gbuf[:, sslot, :],
                        x_self[gg * 128:(gg + 1) * 128, :])

                # batched S build + w-scale for whole SG
                smat = sel_tp.tile([128, max_sg_tiles, 128], BF16)
                nc.vector.tensor_tensor(
                    out=smat[:, :ntile, :],
                    in0=sb_colrel[:, base:base + ntile, None].to_broadcast(
                        [128, ntile, 128]),
                    in1=sb_iota[:, None, :].to_broadcast([128, ntile, 128]),
                    op=mybir.AluOpType.is_equal)
                nc.vector.tensor_tensor(
                    out=gbuf[:, :ntile, :],
                    in0=gbuf[:, :ntile, :],
                    in1=sb_w[:, base:base + ntile, None].to_broadcast(
                        [128, ntile, 128]),
                    op=mybir.AluOpType.mult)

                for gg in range(s, e):
                    ytb = yts[gg % 2]
                    pyt = pyt_tp.tile([128, 128], FP32, space="PSUM")
                    slots = (
                        list(range(meta["slot_lo"][gg] - base,
                                   meta["slot_lo"][gg] - base + meta["n_lo"][gg]))
                        + list(range(meta["slot_hi"][gg] - base,
                                     meta["slot_hi"][gg] - base + meta["n_hi"][gg]))
                        + [meta["slot_self"][gg] - base])
                    for i, sl in enumerate(slots):
                        nc.tensor.matmul(
                            out=pyt[:], lhsT=gbuf[:, sl, :],
                            rhs=smat[:, sl, :],
                            start=(i == 0), stop=(i == len(slots) - 1))
                    for b in range(B):
                        nc.scalar.activation(
                            out=ytb[b][0:24, :],
                            in_=pyt[b * 32: b * 32 + 24, :],
                            func=mybir.ActivationFunctionType.Copy)

                    zc = zh_tp.tile([128, B, T * O], BF16)
                    ht = zh_tp.tile([128, B, T * O], BF16)
                    for b in range(B):
                        pz = pzh_tp.tile([128, T * O], FP32, space="PSUM")
                        ph = pzh_tp.tile([128, T * O], FP32, space="PSUM")
                        nc.tensor.matmul(out=pz[:], lhsT=ytb[b][:, :],
                                         rhs=sb_wz[:], start=True, stop=True)
                        nc.tensor.matmul(out=ph[:], lhsT=ytb[b][:, :],
                                         rhs=sb_wh[:], start=True, stop=True)
                        nc.scalar.activation(
                            out=zc[:, b, :], in_=pz[:],
                            func=mybir.ActivationFunctionType.Sigmoid)
                        nc.scalar.activation(
                            out=ht[:, b, :], in_=ph[:],
                            func=mybir.ActivationFunctionType.Tanh)
                    # comb = zc*pv -> *ht -> tree-reduce over t
                    nc.vector.tensor_tensor(
                        out=zc[:], in0=zc[:],
                        in1=sb_pv[:, None, :].to_broadcast([128, B, T * O]),
                        op=mybir.AluOpType.mult)
                    nc.vector.tensor_tensor(out=zc[:], in0=zc[:], in1=ht[:],
                                            op=mybir.AluOpType.mult)
                    nc.vector.tensor_tensor(
                        out=zc[:, :, 0:192], in0=zc[:, :, 0:192],
                        in1=zc[:, :, 192:384], op=mybir.AluOpType.add)
                    nc.vector.tensor_tensor(
                        out=zc[:, :, 0:96], in0=zc[:, :, 0:96],
                        in1=zc[:, :, 96:192], op=mybir.AluOpType.add)
                    nc.vector.tensor_tensor(
                        out=zc[:, :, 0:32], in0=zc[:, :, 0:32],
                        in1=zc[:, :, 32:64], op=mybir.AluOpType.add)
                    osb = osb_tp.tile([128, B, O], FP32)
                    nc.vector.tensor_tensor(
                        out=osb[:], in0=zc[:, :, 0:32], in1=zc[:, :, 64:96],
                        op=mybir.AluOpType.add)
                    for b in range(B):
                        nc.sync.dma_start(
                            out[b, gg * 128:(gg + 1) * 128, :], osb[:, b, :])
    nc.compile()
    return nc


def kernel(**inputs):
    X = np.asarray(inputs["X"])
    key = "k"
    meta, consts, per_core_in = _build_host(
        X, inputs["edge_index"], inputs["edge_weight"],
        inputs["W_z"], inputs["b_z"], inputs["W_h"], inputs["b_h"],
        inputs["lin_z_w"], inputs["lin_z_b"], inputs["lin_h_w"],
        inputs["lin_h_b"], inputs["attention"])

    if key not in _CACHE:
        _CACHE[key] = _build_graph(meta)
    nc = _CACHE[key]

    in_maps = []
    for c in range(NC):
        m = dict(consts)
        m.update(per_core_in[c])
        in_maps.append({k: np.ascontiguousarray(v) for k, v in m.items()})
    res = run_bass_kernel_spmd(nc, in_maps, core_ids=list(range(NC)))
    _CACHE["last_results"] = res
    full = np.empty((B, N, O), dtype=np.float32)
    for c in range(NC):
        full[:, c * NPC:(c + 1) * NPC, :] = res.results[c]["out"][:, :NPC, :]
    return full
